# revision 1
# baseline (speedup 1.0000x reference)
"""CrossTableAttention Trainium2 kernel (8-core SPMD, batch-sharded).

Math (per table t, row b):
  rel_w[t,r]   = sigmoid(rel_embs[t,r] . w_rel + b_rel)          (host, tiny)
  Qp[t]        = emb[t] @ Wq.T (+bq)                              [B, D]
  Kb[j]        = emb[j] @ Wk.T        (bias bk is softmax-invariant -> dropped)
  Vb[j]        = emb[j] @ Wv.T        (bias bv folded into output bias)
  score[t,b,h,r] = rel_w[t,r] * (Qp[t,b,h,:] . Kb[j_r,b,h,:]) / sqrt(DH)
  attn         = softmax_r(score);  P = attn * rel_w
  ctx[t,b]     = sum_r P[t,b,h,r] * Vb[j_r,b,h,:]
  out[t]       = ctx[t] @ Wo.T + (Wo @ bv + bo)

Key algebraic optimization: K/V projections are computed per *table* (16) instead
of per (table, relation) gather (128) - the relation weight is a scalar that
commutes with the linear projection.  5x fewer matmul FLOPs than the reference.

Device layout notes:
 - Activations live as [row(b) on partitions, feature on free] so the attention
   inner products reduce along the free axis (DVE).
 - q/k/v/ctx feature axis is permuted to dh-major (f = dh*16 + h) so the
   per-(b,h) attention-weight broadcast has a step-1 innermost AP dim
   (keeps DVE tensor_tensor in 2x bf16 mode).  Weight matrices are permuted
   host-side to produce/consume this layout directly.
 - scores/P layout per t: [128 b, 128 = r*16 + h] (h innermost).
 - Matmul operands need the contraction dim (d) on partitions; embT is produced
   by bf16 DMA-xbar transposing loads straight from the (host pre-cast) input.
"""

import sys

sys.path.insert(0, "/opt/trn_rl_repo")

import numpy as np
import ml_dtypes

import concourse.bass as bass
import concourse.bacc as bacc_mod
import concourse.mybir as mybir
import concourse.tile as tile
from concourse.bass_utils import run_bass_kernel_spmd

T, B, D, R, H = 16, 1024, 1024, 8, 16
DH = D // H  # 64
NCORES = 8
BC = B // NCORES  # 128 rows per core
KCH = D // 128  # 8 contraction chunks

F32 = mybir.dt.float32
BF16 = mybir.dt.bfloat16
AX = mybir.AxisListType
AF = mybir.ActivationFunctionType

# feature permutation: new f = dh*16 + h  <->  old o = h*64 + dh
_PERM = np.array([(f % H) * DH + f // H for f in range(D)], dtype=np.int64)


def _bcast_free(ap, n, pos):
    """Insert a [step=0, n] broadcast dim into an AP's free dims at `pos`
    (pos counts free dims, 0 = outermost free dim)."""
    new = list(ap.ap)
    new.insert(1 + pos, [0, n])
    return bass.AP(tensor=ap.tensor, offset=ap.offset, ap=new)


def _bcast_part(ap, n=128):
    """Broadcast a (DRAM) AP across n partitions by prepending a [0, n] dim."""
    return bass.AP(tensor=ap.tensor, offset=ap.offset, ap=[[0, n]] + list(ap.ap))


def _structure(rel_idx):
    """Host-side dedup of the (t, j) gather structure."""
    groups = []  # per t: list of (j, r0, [extra r's])
    for t in range(T):
        by_j = {}
        for r in range(R):
            by_j.setdefault(int(rel_idx[t, r]), []).append(r)
        groups.append([(j, rs[0], rs[1:]) for j, rs in sorted(by_j.items())])
    pairs_by_j = {j: [] for j in range(T)}
    for t in range(T):
        for j, r0, extras in groups[t]:
            pairs_by_j[j].append((t, r0))
    # first j (in ascending j processing order) touching each t
    first_j = {}
    for j in range(T):
        for t, _ in pairs_by_j[j]:
            if t not in first_j:
                first_j[t] = j
    # last j touching each t (to schedule ctx output DMA)
    last_j = {}
    for j in range(T):
        for t, _ in pairs_by_j[j]:
            last_j[t] = j
    return groups, pairs_by_j, first_j, last_j


def _build(rel_idx, use_bq, use_bo):
    """Build the SPMD bass program (identical on all cores).

    v2 structure (from v1 trace analysis):
     - Q+K projections interleaved per t (shared stationary embT chunk ->
       LDWEIGHTS amortized over 4 matmuls), Kb fully SBUF-resident so the
       scores phase never back-pressures the PE.
     - score pairs emitted in availability order (sorted by max(t, j)) so the
       DVE trails the projection stream without stalls.
     - per-head dot products via a binary tree of tensor_tensor adds (bf16 2x
       mode for the large levels, fp32 tail) instead of 1x tensor_reduce.
     - V projections recomputed in the ctx phase (keeps PE busy there);
       ctx accumulation adds alternate DVE / GPSIMD.
    """
    groups, pairs_by_j, first_j, last_j = _structure(rel_idx)

    # (t, j, r0, extras) sorted by when both Qp[t] and Kb[j] become available
    pair_sched = []
    for t in range(T):
        for j, r0, extras in groups[t]:
            pair_sched.append((max(t, j), t, j, r0, extras))
    pair_sched.sort(key=lambda x: (x[0], x[1]))
    last_pair_of_t = {}
    for i, (_, t, j, r0, ex) in enumerate(pair_sched):
        last_pair_of_t[t] = i

    nc = bacc_mod.Bacc(None, target_bir_lowering=False, debug=False)
    emb_ext = nc.dram_tensor("emb", [T, BC, D], BF16, kind="ExternalInput")
    wq_ext = nc.dram_tensor("wq", [D, D], BF16, kind="ExternalInput")
    wk_ext = nc.dram_tensor("wk", [D, D], BF16, kind="ExternalInput")
    wv_ext = nc.dram_tensor("wv", [D, D], BF16, kind="ExternalInput")
    wo_ext = nc.dram_tensor("wo", [D, D], BF16, kind="ExternalInput")
    rw_ext = nc.dram_tensor("rw", [T, R * H], F32, kind="ExternalInput")
    if use_bq:
        bq_ext = nc.dram_tensor("bqp", [D], F32, kind="ExternalInput")
    if use_bo:
        bo_ext = nc.dram_tensor("boe", [D], F32, kind="ExternalInput")
    out_ext = nc.dram_tensor("out", [T, BC, D], F32, kind="ExternalOutput")

    with tile.TileContext(nc) as tc:
        with (
            tc.tile_pool(name="consts", bufs=1) as consts,
            tc.tile_pool(name="wpool", bufs=2) as wpool,
            tc.tile_pool(name="big", bufs=1) as big,
            tc.tile_pool(name="embp", bufs=1) as embp,
            tc.tile_pool(name="kball", bufs=1) as kballp,
            tc.tile_pool(name="attn", bufs=1) as attnp,
            tc.tile_pool(name="kv", bufs=3) as kvp,
            tc.tile_pool(name="work", bufs=3) as work,
            tc.tile_pool(name="smalls", bufs=3) as smalls,
            tc.tile_pool(name="outp", bufs=2) as outp,
            tc.tile_pool(name="ctxT", bufs=2) as ctxTp,
            tc.tile_pool(name="psum", bufs=8, space="PSUM") as psum,
            tc.tile_pool(name="dram", bufs=1, space="DRAM") as dramp,
        ):
            ctx_dram = dramp.tile([T, BC, D], BF16)
            # ---- constant + weight loads ----
            rw_full = consts.tile([128, T, R * H], F32)
            nc.gpsimd.dma_start(out=rw_full, in_=_bcast_part(rw_ext[:]))
            if use_bq:
                bq_full = consts.tile([128, D], F32)
                nc.gpsimd.dma_start(out=bq_full, in_=_bcast_part(bq_ext[:]))
            if use_bo:
                bo_full = consts.tile([128, D], F32)
                nc.gpsimd.dma_start(out=bo_full, in_=_bcast_part(bo_ext[:]))

            wq_t = wpool.tile([128, KCH, D], BF16, tag="w")
            nc.gpsimd.dma_start(
                out=wq_t, in_=wq_ext.rearrange("(k p) o -> p k o", p=128)
            )
            wk_t = wpool.tile([128, KCH, D], BF16, tag="w")
            nc.gpsimd.dma_start(
                out=wk_t, in_=wk_ext.rearrange("(k p) o -> p k o", p=128)
            )

            # embT[p, k, t*BC+b] = emb[t, b, k*128+p]  (bf16 xbar transposes,
            # split across both HWDGE sequencers to shorten the startup gate)
            embT = embp.tile([128, KCH, T * BC], BF16)
            for k in range(KCH):
                nc.scalar.dma_start_transpose(
                    out=embT[:, k, :],
                    in_=emb_ext[:, :, k * 128 : (k + 1) * 128].rearrange(
                        "t b d -> (t b) d"
                    ),
                )

            qp_all = big.tile([128, T, D], BF16, tag="qpctx")
            kb_all = kballp.tile([128, T, D], BF16)
            scores = attnp.tile([128, T, R * H], F32)
            p2_all = attnp.tile([128, T, R * H], BF16)

            def emit_pair(t, j, r0, extras):
                """scores[t, r0-block] = per-head dot(Qp[t], Kb[j]) via
                bf16 mul + binary-tree halving over dh (dh-major layout:
                folding dh halves == folding contiguous column halves)."""
                prod = work.tile([128, D], BF16, tag="prod")
                nc.vector.tensor_mul(prod, qp_all[:, t, :], kb_all[:, j, :])
                nc.vector.tensor_add(prod[:, 0:512], prod[:, 0:512], prod[:, 512:1024])
                nc.vector.tensor_add(prod[:, 0:256], prod[:, 0:256], prod[:, 256:512])
                sc32 = smalls.tile([128, 128], F32, tag="sc32")
                nc.vector.tensor_add(sc32, prod[:, 0:128], prod[:, 128:256])
                nc.vector.tensor_add(sc32[:, 0:64], sc32[:, 0:64], sc32[:, 64:128])
                nc.vector.tensor_add(sc32[:, 0:32], sc32[:, 0:32], sc32[:, 32:64])
                dst = scores[:, t, r0 * H : (r0 + 1) * H]
                nc.vector.tensor_add(dst, sc32[:, 0:16], sc32[:, 16:32])
                for rx in extras:
                    nc.vector.tensor_copy(scores[:, t, rx * H : (rx + 1) * H], dst)

            def emit_softmax(t):
                s_t = scores[:, t, :]
                nc.vector.tensor_mul(s_t, s_t, rw_full[:, t, :])  # *= rel_w
                m = smalls.tile([128, H], F32, tag="m")
                nc.vector.reduce_max(
                    out=m, in_=s_t.rearrange("p (r h) -> p h r", h=H), axis=AX.X
                )
                e_t = p2_all[:, t, :]
                nc.vector.tensor_sub(
                    e_t.rearrange("p (r h) -> p r h", h=H),
                    s_t.rearrange("p (r h) -> p r h", h=H),
                    _bcast_free(m, R, 0),
                )
                # exp((s - m) / sqrt(DH)); the 1/8 rides ACT's free affine
                nc.scalar.activation(e_t, e_t, AF.Exp, scale=0.125)
                ssum = smalls.tile([128, H], F32, tag="ssum")
                nc.vector.reduce_sum(
                    out=ssum, in_=e_t.rearrange("p (r h) -> p h r", h=H), axis=AX.X
                )
                inv = smalls.tile([128, H], F32, tag="inv")
                nc.vector.reciprocal(inv, ssum)
                nc.vector.tensor_mul(
                    e_t.rearrange("p (r h) -> p r h", h=H),
                    e_t.rearrange("p (r h) -> p r h", h=H),
                    _bcast_free(inv, R, 0),
                )
                nc.vector.tensor_mul(e_t, e_t, rw_full[:, t, :])  # P = attn*rel_w
                for j, r0, extras in groups[t]:
                    for rx in extras:
                        nc.vector.tensor_add(
                            e_t[:, r0 * H : (r0 + 1) * H],
                            e_t[:, r0 * H : (r0 + 1) * H],
                            e_t[:, rx * H : (rx + 1) * H],
                        )

            # ---- phase 1: Q+K projections (dense PE stream) + trailing scores ----
            next_pair = 0
            for t in range(T):
                psq0 = psum.tile([128, 512], F32, tag="ps", name="ps")
                psq1 = psum.tile([128, 512], F32, tag="ps", name="ps")
                psk0 = psum.tile([128, 512], F32, tag="ps", name="ps")
                psk1 = psum.tile([128, 512], F32, tag="ps", name="ps")
                for k in range(KCH):
                    lhs = embT[:, k, t * BC : (t + 1) * BC]
                    fl = dict(start=(k == 0), stop=(k == KCH - 1))
                    nc.tensor.matmul(psq0, lhs, wq_t[:, k, 0:512], **fl)
                    nc.tensor.matmul(psq1, lhs, wq_t[:, k, 512:1024], **fl)
                    nc.tensor.matmul(psk0, lhs, wk_t[:, k, 0:512], **fl)
                    nc.tensor.matmul(psk1, lhs, wk_t[:, k, 512:1024], **fl)
                nc.scalar.copy(out=qp_all[:, t, 0:512], in_=psq0)
                nc.scalar.copy(out=qp_all[:, t, 512:1024], in_=psq1)
                nc.scalar.copy(out=kb_all[:, t, 0:512], in_=psk0)
                nc.scalar.copy(out=kb_all[:, t, 512:1024], in_=psk1)
                if use_bq:
                    nc.vector.tensor_add(qp_all[:, t, :], qp_all[:, t, :], bq_full)
                # emit score pairs that just became available
                while next_pair < len(pair_sched) and pair_sched[next_pair][0] <= t:
                    _, tp, jp, r0p, exp_ = pair_sched[next_pair]
                    emit_pair(tp, jp, r0p, exp_)
                    if last_pair_of_t[tp] == next_pair:
                        emit_softmax(tp)
                    next_pair += 1

            wv_t = wpool.tile([128, KCH, D], BF16, tag="w")  # reuses wq slot
            nc.gpsimd.dma_start(
                out=wv_t, in_=wv_ext.rearrange("(k p) o -> p k o", p=128)
            )
            wo_t = wpool.tile([128, KCH, D], BF16, tag="w")  # reuses wk slot
            nc.gpsimd.dma_start(
                out=wo_t, in_=wo_ext.rearrange("(k p) o -> p k o", p=128)
            )

            # ---- phase 2: V projections (recomputed) + ctx accumulation ----
            ctx_all = big.tile([128, T, D], BF16, tag="qpctx")  # reuses qp slot
            done_t = set()
            alt = 0
            for j in range(T):
                psv0 = psum.tile([128, 512], F32, tag="ps", name="ps")
                psv1 = psum.tile([128, 512], F32, tag="ps", name="ps")
                for k in range(KCH):
                    lhs = embT[:, k, j * BC : (j + 1) * BC]
                    fl = dict(start=(k == 0), stop=(k == KCH - 1))
                    nc.tensor.matmul(psv0, lhs, wv_t[:, k, 0:512], **fl)
                    nc.tensor.matmul(psv1, lhs, wv_t[:, k, 512:1024], **fl)
                vb = kvp.tile([128, D], BF16, tag="vb")
                nc.scalar.copy(out=vb[:, 0:512], in_=psv0)
                nc.scalar.copy(out=vb[:, 512:1024], in_=psv1)
                for t, r0 in pairs_by_j[j]:
                    pb = _bcast_free(p2_all[:, t, r0 * H : (r0 + 1) * H], DH, 0)
                    vb3 = vb.rearrange("p (dh h) -> p dh h", h=H)
                    c3 = ctx_all[:, t, :].rearrange("p (dh h) -> p dh h", h=H)
                    if t not in done_t:
                        done_t.add(t)
                        nc.vector.tensor_mul(c3, vb3, pb)
                    else:
                        tmp = work.tile([128, D], BF16, tag="ctmp")
                        nc.vector.tensor_mul(
                            tmp.rearrange("p (dh h) -> p dh h", h=H), vb3, pb
                        )
                        eng = nc.vector if alt % 2 == 0 else nc.gpsimd
                        alt += 1
                        eng.tensor_add(ctx_all[:, t, :], ctx_all[:, t, :], tmp)
                # ship finished ctx rows to DRAM scratch (for xbar re-transpose)
                for t in range(T):
                    if last_j[t] == j:
                        nc.sync.dma_start(out=ctx_dram[t], in_=ctx_all[:, t, :])

            # ---- phase 3: output projection ----
            TG = 2  # t-group size for transposing loads
            for tg in range(T // TG):
                ctxT = ctxTp.tile([128, KCH, TG * BC], BF16)
                for k in range(KCH):
                    nc.scalar.dma_start_transpose(
                        out=ctxT[:, k, :],
                        in_=ctx_dram[
                            tg * TG : (tg + 1) * TG, :, k * 128 : (k + 1) * 128
                        ].rearrange("t b d -> (t b) d"),
                    )
                for ti in range(TG):
                    t = tg * TG + ti
                    o_t = outp.tile([128, D], F32)
                    pso0 = psum.tile([128, 512], F32, tag="ps", name="ps")
                    pso1 = psum.tile([128, 512], F32, tag="ps", name="ps")
                    for k in range(KCH):
                        lhs = ctxT[:, k, ti * BC : (ti + 1) * BC]
                        fl = dict(start=(k == 0), stop=(k == KCH - 1))
                        nc.tensor.matmul(pso0, lhs, wo_t[:, k, 0:512], **fl)
                        nc.tensor.matmul(pso1, lhs, wo_t[:, k, 512:1024], **fl)
                    nc.scalar.copy(out=o_t[:, 0:512], in_=pso0)
                    nc.scalar.copy(out=o_t[:, 512:1024], in_=pso1)
                    if use_bo:
                        nc.vector.tensor_add(o_t, o_t, bo_full)
                    nc.sync.dma_start(out=out_ext[t], in_=o_t)

    return nc


_CACHE = {}


def _get_program(rel_idx, use_bq, use_bo):
    key = (rel_idx.tobytes(), use_bq, use_bo)
    if key not in _CACHE:
        nc = _build(rel_idx, use_bq, use_bo)
        nc.finalize()  # runs the bacc passes (reg alloc, wait lowering, ...)
        _CACHE[key] = nc
    return _CACHE[key]


def kernel(
    table_embs,
    rel_embs,
    rel_idx,
    Wq,
    bq,
    Wk,
    bk,
    Wv,
    bv,
    Wo,
    bo,
    w_rel,
    b_rel,
    _trace=False,
):
    table_embs = np.asarray(table_embs, dtype=np.float32)
    rel_embs = np.asarray(rel_embs, dtype=np.float32)
    rel_idx = np.asarray(rel_idx).astype(np.int64)
    Wq, Wk, Wv, Wo = (np.asarray(w, dtype=np.float32) for w in (Wq, Wk, Wv, Wo))
    bq, bk, bv, bo = (np.asarray(b, dtype=np.float32) for b in (bq, bk, bv, bo))
    w_rel = np.asarray(w_rel, dtype=np.float32)
    b_rel = np.asarray(b_rel, dtype=np.float32)

    # ---- host-side tiny prep ----
    rw = 1.0 / (1.0 + np.exp(-(rel_embs @ w_rel + b_rel[0])))  # [T, R] fp32
    rw_full = np.repeat(rw.astype(np.float32), H, axis=1)  # [T, R*H], col=r*16+h
    bf = ml_dtypes.bfloat16
    wq_p = np.ascontiguousarray(Wq.T[:, _PERM], dtype=bf)
    wk_p = np.ascontiguousarray(Wk.T[:, _PERM], dtype=bf)
    wv_p = np.ascontiguousarray(Wv.T[:, _PERM], dtype=bf)
    wo_p = np.ascontiguousarray(Wo.T[_PERM, :], dtype=bf)
    use_bq = bool(np.any(bq))
    bo_eff = Wo @ bv + bo
    use_bo = bool(np.any(bo_eff))
    bq_p = np.ascontiguousarray(bq[_PERM], dtype=np.float32)

    nc = _get_program(rel_idx, use_bq, use_bo)

    in_maps = []
    for c in range(NCORES):
        m = {
            "emb": np.ascontiguousarray(
                table_embs[:, c * BC : (c + 1) * BC, :], dtype=bf
            ),
            "wq": wq_p,
            "wk": wk_p,
            "wv": wv_p,
            "wo": wo_p,
            "rw": rw_full,
        }
        if use_bq:
            m["bqp"] = bq_p
        if use_bo:
            m["boe"] = bo_eff.astype(np.float32)
        in_maps.append(m)

    res = run_bass_kernel_spmd(nc, in_maps, list(range(NCORES)), trace=_trace)
    out = np.empty((T, B, D), dtype=np.float32)
    for c in range(NCORES):
        out[:, c * BC : (c + 1) * BC, :] = res.results[c]["out"]
    if _trace:
        kernel._last_results = res
    return out



# revision 9
# speedup vs baseline: 1.6020x; 1.6020x over previous
"""CrossTableAttention Trainium2 kernel v3 (8-core SPMD, batch-sharded).

Math (per table t, row b, head h, relation slot s):
  rw[t,r]    = sigmoid(rel_embs[t,r] . w_rel + b_rel)                (host)
  qT[f, tb]  = (emb @ Wq.T).T   (feature-on-partition layout, dh-major perm)
  kT, vT     likewise (bk is softmax-shift-invariant -> dropped; bv folds to
               a host-side output addend Wo@bv since sum_r attn = 1)
  score[b, (s,h)] = 0.125*rw[t,s] * sum_f qT[f,tb] kT[f,jb]   (PE matmul with
               moving operand selrw = 0.125*rw*onehot(h), contraction over f)
  e = exp(score);  attn = e / sum_s e;  P = attn * rw
  ctx[f, b]  = sum_s P[b,(s,h(f))] * vT[j_s][f, b]
  out        = (ctx.T @ Wo.T).T (+ host bias)

v3 structure (from v2 trace analysis: PE HAM-throttled by idle gaps, DVE
broadcast-AP muls at 1x, GPSIMD adds 4x slow):
 - everything activation-side lives in [feature-part, row-free] layout, so
   projections chain with no DMA transposes (host pre-transposes emb once).
 - the per-(b,h) score dot is: DVE elementwise mul + 3 binary folds to one
   128-part chunk, then ONE PE matmul per (t,j) pair with a tiny stationary
   (the folded prod) and rw-scaled one-hot moving operand -> scores land
   row-major [b, (s,h)] in PSUM; exp'd straight out of PSUM by ACT.
 - softmax sum/normalize on DVE (small [128,128] ops), P transposed once per
   table on the PE; per-pair duplicate-merge + head-broadcast of P is a
   single PE matmul against a precomputed 0/1 selection stationary.
 - ctx accumulation: flat bf16 DVE mul/add streams (2x mode, no strided
   broadcast inner dims).
 - PE stream (4 projections + tiny attention MMs) is kept dense so the HAM
   clock gate stays at 8/8.
"""

import sys

sys.path.insert(0, "/opt/trn_rl_repo")

import numpy as np
import ml_dtypes

import concourse.bass as bass
import concourse.bacc as bacc_mod
import concourse.mybir as mybir
import concourse.tile as tile
from concourse.bass_utils import run_bass_kernel_spmd

T, B, D, R, H = 16, 1024, 1024, 8, 16
DH = D // H  # 64
NCORES = 8
BC = B // NCORES  # 128 rows per core
KCH = D // 128  # 8 feature chunks
TB = T * BC  # 2048 (t,b) columns per core

F32 = mybir.dt.float32
BF16 = mybir.dt.bfloat16
AX = mybir.AxisListType
AF = mybir.ActivationFunctionType

# feature permutation: new f = dh*16 + h  <->  old o = h*64 + dh
_PERM = np.array([(f % H) * DH + f // H for f in range(D)], dtype=np.int64)


def _bcast_free(ap, n, pos):
    """Insert a [step=0, n] broadcast dim into an AP's free dims at `pos`."""
    new = list(ap.ap)
    new.insert(1 + pos, [0, n])
    return bass.AP(tensor=ap.tensor, offset=ap.offset, ap=new)


def _structure(rel_idx):
    """Slot assignment: per t, unique j's get consecutive slot runs."""
    pairs = []  # (t, j, s0, m)
    slot_r = np.zeros((T, R), np.int64)
    for t in range(T):
        by_j = {}
        for r in range(R):
            by_j.setdefault(int(rel_idx[t, r]), []).append(r)
        s0 = 0
        for j, rs in sorted(by_j.items()):
            pairs.append((t, j, s0, len(rs)))
            for i, r in enumerate(rs):
                slot_r[t, s0 + i] = r
            s0 += len(rs)
    combos = sorted({(s0, m) for (_, _, s0, m) in pairs})
    merge_idx = {c: i for i, c in enumerate(combos)}
    return pairs, slot_r, merge_idx


def _build(rel_idx, use_bq, use_bo):
    pairs, slot_r, merge_idx = _structure(rel_idx)
    ncmb = len(merge_idx)

    nc = bacc_mod.Bacc(None, target_bir_lowering=False, debug=False)
    emb_ext = nc.dram_tensor("emb", [KCH, 128, TB], BF16, kind="ExternalInput")
    wq_ext = nc.dram_tensor("wq", [D, D], BF16, kind="ExternalInput")
    wk_ext = nc.dram_tensor("wk", [D, D], BF16, kind="ExternalInput")
    wv_ext = nc.dram_tensor("wv", [D, D], BF16, kind="ExternalInput")
    wo_ext = nc.dram_tensor("wo", [D, D], BF16, kind="ExternalInput")
    selrw_ext = nc.dram_tensor("selrw", [128, T * 128], BF16, kind="ExternalInput")
    rwrep_ext = nc.dram_tensor("rwrep", [128, T * 128], BF16, kind="ExternalInput")
    merge_ext = nc.dram_tensor("mrg", [128, ncmb * 128], BF16, kind="ExternalInput")
    ident_ext = nc.dram_tensor("ident", [128, 128], BF16, kind="ExternalInput")
    if use_bq:
        bq_ext = nc.dram_tensor("bqp", [128, KCH], F32, kind="ExternalInput")
    if use_bo:
        bo_ext = nc.dram_tensor("boe", [128, KCH], F32, kind="ExternalInput")
    out_ext = nc.dram_tensor("out", [KCH, 128, TB], F32, kind="ExternalOutput")

    with tile.TileContext(nc) as tc:
        with (
            tc.tile_pool(name="consts", bufs=1) as consts,
            tc.tile_pool(name="wpool", bufs=2) as wpool,
            tc.tile_pool(name="embp", bufs=1) as embp,
            tc.tile_pool(name="qp", bufs=1) as qp,
            tc.tile_pool(name="kp", bufs=1) as kp,
            tc.tile_pool(name="vp", bufs=1) as vp,
            tc.tile_pool(name="attn", bufs=1) as attnp,
            tc.tile_pool(name="prodp", bufs=3) as prodp,
            tc.tile_pool(name="pbsb", bufs=3) as pbsbp,
            tc.tile_pool(name="smalls", bufs=3) as smalls,
            tc.tile_pool(name="outp", bufs=3) as outp,
            tc.tile_pool(name="projps", bufs=3, space="PSUM") as projps,
            tc.tile_pool(name="scps", bufs=2, space="PSUM") as scps,
            tc.tile_pool(name="pbps", bufs=2, space="PSUM") as pbps,
            tc.tile_pool(name="ptps", bufs=1, space="PSUM") as ptps,
        ):
            # ---- constant + weight + input loads ----
            selrw = consts.tile([128, T * 128], BF16)
            nc.gpsimd.dma_start(out=selrw, in_=selrw_ext[:])
            rwrep = consts.tile([128, T * 128], BF16)
            nc.gpsimd.dma_start(out=rwrep, in_=rwrep_ext[:])
            mergeT = consts.tile([128, ncmb * 128], BF16)
            nc.gpsimd.dma_start(out=mergeT, in_=merge_ext[:])
            ident = consts.tile([128, 128], BF16)
            nc.gpsimd.dma_start(out=ident, in_=ident_ext[:])
            if use_bq:
                bqp = consts.tile([128, KCH], F32)
                nc.gpsimd.dma_start(out=bqp, in_=bq_ext[:])
            if use_bo:
                boe = consts.tile([128, KCH], F32)
                nc.gpsimd.dma_start(out=boe, in_=bo_ext[:])

            wq_t = wpool.tile([128, KCH, D], BF16, tag="w")
            nc.gpsimd.dma_start(
                out=wq_t, in_=wq_ext.rearrange("(k p) o -> p k o", p=128)
            )
            wk_t = wpool.tile([128, KCH, D], BF16, tag="w")
            nc.gpsimd.dma_start(
                out=wk_t, in_=wk_ext.rearrange("(k p) o -> p k o", p=128)
            )

            # embT[p, k, tb] — host pre-transposed; split loads per (k, half)
            embT = embp.tile([128, KCH, TB], BF16)
            for half in range(2):
                for k in range(KCH):
                    eng = nc.sync if (k % 2 == 0) else nc.scalar
                    eng.dma_start(
                        out=embT[:, k, half * 1024 : (half + 1) * 1024],
                        in_=emb_ext[k, :, half * 1024 : (half + 1) * 1024],
                    )

            qT = qp.tile([128, KCH, TB], BF16)
            kT = kp.tile([128, KCH, TB], BF16, tag="kc")
            vT = vp.tile([128, KCH, TB], BF16)
            e_all = attnp.tile([128, T, 128], BF16)
            P_all = attnp.tile([128, T, 128], BF16)
            pt_all = attnp.tile([128, T, 128], BF16)

            def proj_cstep(w_t, dst_all, half, c, bias_t):
                ps0 = projps.tile([128, 512], F32, tag="pp", name="pp")
                ps1 = projps.tile([128, 512], F32, tag="pp", name="pp")
                base = half * 1024
                for k in range(KCH):
                    lhsT = w_t[:, k, c * 128 : (c + 1) * 128]
                    fl = dict(start=(k == 0), stop=(k == KCH - 1))
                    nc.tensor.matmul(ps0, lhsT, embT[:, k, base : base + 512], **fl)
                    nc.tensor.matmul(
                        ps1, lhsT, embT[:, k, base + 512 : base + 1024], **fl
                    )
                d0 = dst_all[:, c, base : base + 512]
                d1 = dst_all[:, c, base + 512 : base + 1024]
                if bias_t is not None:
                    nc.scalar.activation(d0, ps0, AF.Identity, bias=bias_t[:, c])
                    nc.scalar.activation(d1, ps1, AF.Identity, bias=bias_t[:, c])
                else:
                    nc.scalar.copy(out=d0, in_=ps0)
                    nc.scalar.copy(out=d1, in_=ps1)

            def emit_pair(t, j, s0, m):
                """score row-block for pair (t,j): DVE fold + 1 PE matmul."""
                prod = prodp.tile([128, KCH * 128], BF16, tag="prod")
                pv = prod.rearrange("p (k b) -> p k b", b=128)
                nc.vector.tensor_mul(
                    pv,
                    qT[:, :, t * 128 : (t + 1) * 128],
                    kT[:, :, j * 128 : (j + 1) * 128],
                )
                nc.vector.tensor_add(
                    prod[:, 0:512], prod[:, 0:512], prod[:, 512:1024]
                )
                nc.vector.tensor_add(prod[:, 0:256], prod[:, 0:256], prod[:, 256:512])
                nc.vector.tensor_add(prod[:, 0:128], prod[:, 0:128], prod[:, 128:256])
                sc = scps.tile([128, 48], F32, tag="sc", name="sc")
                nc.tensor.matmul(
                    sc[:, 0 : m * 16],
                    prod[:, 0:128],
                    selrw[:, t * 128 + s0 * 16 : t * 128 + (s0 + m) * 16],
                    start=True,
                    stop=True,
                )
                nc.scalar.activation(
                    e_all[:, t, s0 * 16 : (s0 + m) * 16], sc[:, 0 : m * 16], AF.Exp
                )

            def emit_chain(t):
                """softmax normalize + P transpose for table t."""
                ssum = smalls.tile([128, H], F32, tag="ssum")
                nc.vector.reduce_sum(
                    out=ssum,
                    in_=e_all[:, t, :].rearrange("p (s h) -> p h s", h=H),
                    axis=AX.X,
                )
                inv = smalls.tile([128, H], F32, tag="inv")
                nc.vector.reciprocal_approx_fast(out=inv, in_=ssum)
                # P = e * rw(slot) * inv(head)
                nc.vector.tensor_mul(
                    P_all[:, t, :], e_all[:, t, :], rwrep[:, t * 128 : (t + 1) * 128]
                )
                nc.vector.tensor_mul(
                    P_all[:, t, :].rearrange("p (s h) -> p s h", h=H),
                    P_all[:, t, :].rearrange("p (s h) -> p s h", h=H),
                    _bcast_free(inv[:], R, 0),
                )
                ptp = ptps.tile([128, 128], BF16, tag="pt", name="pt")
                nc.tensor.transpose(ptp, P_all[:, t, :], ident)
                nc.scalar.copy(out=pt_all[:, t, :], in_=ptp)

            ctx_first = set()

            def emit_ctx(t, j, s0, m, ctx_all):
                """ctx[t] += merged/broadcast P ⊙ vT[j]."""
                idx = merge_idx[(s0, m)]
                pb = pbps.tile([128, 128], F32, tag="pb", name="pb")
                nc.tensor.matmul(
                    pb,
                    mergeT[:, idx * 128 : (idx + 1) * 128],
                    pt_all[:, t, :],
                    start=True,
                    stop=True,
                )
                pbs = pbsbp.tile([128, 128], BF16, tag="pbs")
                nc.scalar.copy(out=pbs, in_=pb)
                vs = vT[:, :, j * 128 : (j + 1) * 128]
                dst = ctx_all[:, :, t * 128 : (t + 1) * 128]
                if t not in ctx_first:
                    ctx_first.add(t)
                    nc.vector.tensor_mul(dst, vs, _bcast_free(pbs[:], KCH, 0))
                else:
                    tmp = prodp.tile([128, KCH * 128], BF16, tag="prod")
                    tv = tmp.rearrange("p (k b) -> p k b", b=128)
                    nc.vector.tensor_mul(tv, vs, _bcast_free(pbs[:], KCH, 0))
                    nc.vector.tensor_add(dst, dst, tv)

            def emit_o_costep(wo_t, ctx_all, g, co):
                ps = projps.tile([128, 512], F32, tag="pp", name="pp")
                for ci in range(KCH):
                    nc.tensor.matmul(
                        ps,
                        wo_t[:, ci, co * 128 : (co + 1) * 128],
                        ctx_all[:, ci, g * 512 : (g + 1) * 512],
                        start=(ci == 0),
                        stop=(ci == KCH - 1),
                    )
                ob = outp.tile([128, 512], F32)
                if use_bo:
                    nc.scalar.activation(ob, ps, AF.Identity, bias=boe[:, co])
                else:
                    nc.scalar.copy(out=ob, in_=ps)
                nc.sync.dma_start(
                    out=out_ext[co, :, g * 512 : (g + 1) * 512], in_=ob
                )

            # ---------- wave scheduler ----------
            q_done = [False, False]
            k_done = [False, False]
            v_done = [False, False]
            pairs_left_of_t = {t: 0 for t in range(T)}
            for (t, j, s0, m) in pairs:
                pairs_left_of_t[t] += 1
            chain_done = set()
            all_pairs_emitted = [False]
            todo_pairs = list(pairs)
            todo_ctx = []  # filled as chains complete
            ctx_left_of_t = {t: 0 for t in range(T)}
            for (t, j, s0, m) in pairs:
                ctx_left_of_t[t] += 1
            ctx_all_holder = [None]

            def pair_ready(p):
                t, j, s0, m = p
                return q_done[t // 8] and k_done[j // 8]

            def ctx_ready(c):
                t, j, s0, m = c
                return (
                    all_pairs_emitted[0]
                    and t in chain_done
                    and v_done[j // 8]
                    and ctx_all_holder[0] is not None
                )

            def flush(budget):
                n = 0
                i = 0
                while i < len(todo_pairs) and n < budget:
                    p = todo_pairs[i]
                    if pair_ready(p):
                        todo_pairs.pop(i)
                        emit_pair(*p)
                        n += 1
                        t = p[0]
                        pairs_left_of_t[t] -= 1
                        if pairs_left_of_t[t] == 0:
                            emit_chain(t)
                            chain_done.add(t)
                            for c in pairs:
                                if c[0] == t:
                                    todo_ctx.append(c)
                    else:
                        i += 1
                if not todo_pairs:
                    all_pairs_emitted[0] = True
                i = 0
                while i < len(todo_ctx) and n < budget:
                    c = todo_ctx[i]
                    if ctx_ready(c):
                        todo_ctx.pop(i)
                        emit_ctx(*c, ctx_all_holder[0])
                        n += 1
                        ctx_left_of_t[c[0]] -= 1
                    else:
                        i += 1
                return n

            # ---------- main schedule ----------
            phase_list = [
                ("q", 0, qT), ("k", 0, kT), ("q", 1, qT), ("k", 1, kT),
                ("v", 0, vT), ("v", 1, vT),
            ]
            wv_t = wo_t = None
            for (pname, half, dst_all) in phase_list:
                w_t = {"q": wq_t, "k": wk_t, "v": wv_t}[pname]
                bias_t = bqp if (pname == "q" and use_bq) else None
                for c in range(KCH):
                    proj_cstep(w_t, dst_all, half, c, bias_t)
                    flush(3)
                if pname == "q":
                    q_done[half] = True
                    if half == 1:
                        # wq dead -> prefetch wv into its slot (overlaps k,1)
                        wv_t = wpool.tile([128, KCH, D], BF16, tag="w")
                        nc.gpsimd.dma_start(
                            out=wv_t,
                            in_=wv_ext.rearrange("(k p) o -> p k o", p=128),
                        )
                elif pname == "k":
                    k_done[half] = True
                    if half == 1:
                        # wk dead -> prefetch wo into its slot (overlaps v)
                        wo_t = wpool.tile([128, KCH, D], BF16, tag="w")
                        nc.gpsimd.dma_start(
                            out=wo_t,
                            in_=wo_ext.rearrange("(k p) o -> p k o", p=128),
                        )
                else:
                    v_done[half] = True

            # drain every remaining score pair (kT's last readers), then let
            # ctx_all reuse the kT slot
            while todo_pairs:
                if flush(8) == 0:
                    raise RuntimeError("scheduler stuck draining pairs")
            ctx_tile = kp.tile([128, KCH, TB], BF16, tag="kc", name="ctx")
            ctx_all_holder[0] = ctx_tile
            ctx_all = ctx_all_holder[0]
            for g in range(T // 4):
                tables = range(4 * g, 4 * g + 4)
                guard = 0
                while any(ctx_left_of_t[t] > 0 for t in tables):
                    made = flush(6)
                    guard += 1
                    if made == 0 and guard > 1000:
                        raise RuntimeError(
                            f"scheduler stuck: g={g} "
                            f"{[(t, ctx_left_of_t[t]) for t in tables]}"
                        )
                for co in range(KCH):
                    emit_o_costep(wo_t, ctx_all, g, co)
                    flush(3)
            while todo_pairs or todo_ctx:
                if flush(8) == 0:
                    raise RuntimeError("scheduler stuck at tail")

    return nc


_CACHE = {}


def _get_program(rel_idx, use_bq, use_bo):
    key = (rel_idx.tobytes(), use_bq, use_bo)
    if key not in _CACHE:
        nc = _build(rel_idx, use_bq, use_bo)
        nc.finalize()
        _CACHE[key] = nc
    return _CACHE[key]


def kernel(
    table_embs,
    rel_embs,
    rel_idx,
    Wq,
    bq,
    Wk,
    bk,
    Wv,
    bv,
    Wo,
    bo,
    w_rel,
    b_rel,
    _trace=False,
):
    table_embs = np.asarray(table_embs, dtype=np.float32)
    rel_embs = np.asarray(rel_embs, dtype=np.float32)
    rel_idx = np.asarray(rel_idx).astype(np.int64)
    Wq, Wk, Wv, Wo = (np.asarray(w, dtype=np.float32) for w in (Wq, Wk, Wv, Wo))
    bq, bk, bv, bo = (np.asarray(b, dtype=np.float32) for b in (bq, bk, bv, bo))
    w_rel = np.asarray(w_rel, dtype=np.float32)
    b_rel = np.asarray(b_rel, dtype=np.float32)

    pairs, slot_r, merge_idx = _structure(rel_idx)
    ncmb = len(merge_idx)

    # ---- host-side prep ----
    rw = 1.0 / (1.0 + np.exp(-(rel_embs @ w_rel + b_rel[0])))  # [T, R] fp32
    bf = ml_dtypes.bfloat16
    wq_p = np.ascontiguousarray(Wq.T[:, _PERM], dtype=bf)
    wk_p = np.ascontiguousarray(Wk.T[:, _PERM], dtype=bf)
    wv_p = np.ascontiguousarray(Wv.T[:, _PERM], dtype=bf)
    wo_p = np.ascontiguousarray(Wo.T[_PERM, :], dtype=bf)

    # selrw[p, t*128 + s*16+h] = 0.125*rw[t, slot_r[t,s]] * (p%16==h)
    # rwrep[p, t*128 + s*16+h] = rw[t, slot_r[t,s]]
    rw_slot = np.take_along_axis(rw, slot_r, axis=1)  # [T, S=R]
    pmod = np.arange(128) % 16
    onehot = (pmod[:, None] == np.arange(16)[None, :]).astype(np.float32)  # [128,16]
    selrw = np.zeros((128, T, R, 16), np.float32)
    rwrep = np.zeros((128, T, R, 16), np.float32)
    for t in range(T):
        for s in range(R):
            selrw[:, t, s, :] = 0.125 * rw_slot[t, s] * onehot
            rwrep[:, t, s, :] = rw_slot[t, s]
    selrw = np.ascontiguousarray(selrw.reshape(128, T * 128), dtype=bf)
    rwrep = np.ascontiguousarray(rwrep.reshape(128, T * 128), dtype=bf)

    # mergeT[p=(s,h), idx*128 + i] = 1{s0<=s<s0+m} * 1{i%16 == h}
    mrg = np.zeros((128, ncmb, 128), np.float32)
    smod = np.arange(128) // 16  # s of partition
    hmod = np.arange(128) % 16  # h of partition
    for (s0, m), idx in merge_idx.items():
        mask = ((smod >= s0) & (smod < s0 + m)).astype(np.float32)  # [128]
        mrg[:, idx, :] = mask[:, None] * (hmod[:, None] == pmod[None, :])
    mrg = np.ascontiguousarray(mrg.reshape(128, ncmb * 128), dtype=bf)
    ident = np.eye(128, dtype=bf)

    use_bq = bool(np.any(bq))
    bo_eff = Wo @ bv + bo
    use_bo = bool(np.any(bo_eff))
    bqp = np.ascontiguousarray(bq[_PERM].reshape(KCH, 128).T, dtype=np.float32)
    boe = np.ascontiguousarray(bo_eff.reshape(KCH, 128).T, dtype=np.float32)

    nc = _get_program(rel_idx, use_bq, use_bo)

    in_maps = []
    for c in range(NCORES):
        e = table_embs[:, c * BC : (c + 1) * BC, :]  # [T, BC, D]
        embT = np.ascontiguousarray(
            e.transpose(2, 0, 1).reshape(KCH, 128, TB), dtype=bf
        )
        m = {
            "emb": embT,
            "wq": wq_p,
            "wk": wk_p,
            "wv": wv_p,
            "wo": wo_p,
            "selrw": selrw,
            "rwrep": rwrep,
            "mrg": mrg,
            "ident": ident,
        }
        if use_bq:
            m["bqp"] = bqp
        if use_bo:
            m["boe"] = boe
        in_maps.append(m)

    res = run_bass_kernel_spmd(nc, in_maps, list(range(NCORES)), trace=_trace)
    out = np.empty((T, B, D), dtype=np.float32)
    for c in range(NCORES):
        o = res.results[c]["out"]  # [KCH, 128, TB]
        out[:, c * BC : (c + 1) * BC, :] = (
            o.reshape(D, T, BC).transpose(1, 2, 0)
        )
    if _trace:
        kernel._last_results = res
    return out


# revision 12
# speedup vs baseline: 1.6323x; 1.0189x over previous
"""CrossTableAttention Trainium2 kernel v3 (8-core SPMD, batch-sharded).

Math (per table t, row b, head h, relation slot s):
  rw[t,r]    = sigmoid(rel_embs[t,r] . w_rel + b_rel)                (host)
  qT[f, tb]  = (emb @ Wq.T).T   (feature-on-partition layout, dh-major perm)
  kT, vT     likewise (bk is softmax-shift-invariant -> dropped; bv folds to
               a host-side output addend Wo@bv since sum_r attn = 1)
  score[b, (s,h)] = 0.125*rw[t,s] * sum_f qT[f,tb] kT[f,jb]   (PE matmul with
               moving operand selrw = 0.125*rw*onehot(h), contraction over f)
  e = exp(score);  attn = e / sum_s e;  P = attn * rw
  ctx[f, b]  = sum_s P[b,(s,h(f))] * vT[j_s][f, b]
  out        = (ctx.T @ Wo.T).T (+ host bias)

v3 structure (from v2 trace analysis: PE HAM-throttled by idle gaps, DVE
broadcast-AP muls at 1x, GPSIMD adds 4x slow):
 - everything activation-side lives in [feature-part, row-free] layout, so
   projections chain with no DMA transposes (host pre-transposes emb once).
 - the per-(b,h) score dot is: DVE elementwise mul + 3 binary folds to one
   128-part chunk, then ONE PE matmul per (t,j) pair with a tiny stationary
   (the folded prod) and rw-scaled one-hot moving operand -> scores land
   row-major [b, (s,h)] in PSUM; exp'd straight out of PSUM by ACT.
 - softmax sum/normalize on DVE (small [128,128] ops), P transposed once per
   table on the PE; per-pair duplicate-merge + head-broadcast of P is a
   single PE matmul against a precomputed 0/1 selection stationary.
 - ctx accumulation: flat bf16 DVE mul/add streams (2x mode, no strided
   broadcast inner dims).
 - PE stream (4 projections + tiny attention MMs) is kept dense so the HAM
   clock gate stays at 8/8.
"""

import sys

sys.path.insert(0, "/opt/trn_rl_repo")

import numpy as np
import ml_dtypes

import concourse.bass as bass
import concourse.bacc as bacc_mod
import concourse.mybir as mybir
import concourse.tile as tile
from concourse.bass_utils import run_bass_kernel_spmd

T, B, D, R, H = 16, 1024, 1024, 8, 16
DH = D // H  # 64
NCORES = 8
BC = B // NCORES  # 128 rows per core
KCH = D // 128  # 8 feature chunks
TB = T * BC  # 2048 (t,b) columns per core

F32 = mybir.dt.float32
BF16 = mybir.dt.bfloat16
AX = mybir.AxisListType
AF = mybir.ActivationFunctionType

# feature permutation: new f = dh*16 + h  <->  old o = h*64 + dh
_PERM = np.array([(f % H) * DH + f // H for f in range(D)], dtype=np.int64)


def _bcast_free(ap, n, pos):
    """Insert a [step=0, n] broadcast dim into an AP's free dims at `pos`."""
    new = list(ap.ap)
    new.insert(1 + pos, [0, n])
    return bass.AP(tensor=ap.tensor, offset=ap.offset, ap=new)


def _structure(rel_idx):
    """Slot assignment: per t, unique j's get consecutive slot runs."""
    pairs = []  # (t, j, s0, m)
    slot_r = np.zeros((T, R), np.int64)
    for t in range(T):
        by_j = {}
        for r in range(R):
            by_j.setdefault(int(rel_idx[t, r]), []).append(r)
        s0 = 0
        for j, rs in sorted(by_j.items()):
            pairs.append((t, j, s0, len(rs)))
            for i, r in enumerate(rs):
                slot_r[t, s0 + i] = r
            s0 += len(rs)
    combos = sorted({(s0, m) for (_, _, s0, m) in pairs})
    merge_idx = {c: i for i, c in enumerate(combos)}
    return pairs, slot_r, merge_idx


def _build(rel_idx, use_bq, use_bo):
    pairs, slot_r, merge_idx = _structure(rel_idx)
    ncmb = len(merge_idx)

    nc = bacc_mod.Bacc(None, target_bir_lowering=False, debug=False)
    emb_ext = nc.dram_tensor("emb", [KCH, 128, TB], BF16, kind="ExternalInput")
    wq_ext = nc.dram_tensor("wq", [D, D], BF16, kind="ExternalInput")
    wk_ext = nc.dram_tensor("wk", [D, D], BF16, kind="ExternalInput")
    wv_ext = nc.dram_tensor("wv", [D, D], BF16, kind="ExternalInput")
    wo_ext = nc.dram_tensor("wo", [D, D], BF16, kind="ExternalInput")
    selrw_ext = nc.dram_tensor("selrw", [128, T * 128], BF16, kind="ExternalInput")
    rwrep_ext = nc.dram_tensor("rwrep", [128, T * 128], BF16, kind="ExternalInput")
    merge_ext = nc.dram_tensor("mrg", [128, ncmb * 128], BF16, kind="ExternalInput")
    ident_ext = nc.dram_tensor("ident", [128, 128], BF16, kind="ExternalInput")
    if use_bq:
        bq_ext = nc.dram_tensor("bqp", [128, KCH], F32, kind="ExternalInput")
    if use_bo:
        bo_ext = nc.dram_tensor("boe", [128, KCH], F32, kind="ExternalInput")
    out_ext = nc.dram_tensor("out", [KCH, 128, TB], F32, kind="ExternalOutput")

    with tile.TileContext(nc) as tc:
        with (
            tc.tile_pool(name="consts", bufs=1) as consts,
            tc.tile_pool(name="wpool", bufs=2) as wpool,
            tc.tile_pool(name="embp", bufs=1) as embp,
            tc.tile_pool(name="qp", bufs=1) as qp,
            tc.tile_pool(name="kp", bufs=1) as kp,
            tc.tile_pool(name="vp", bufs=1) as vp,
            tc.tile_pool(name="attn", bufs=1) as attnp,
            tc.tile_pool(name="prodp", bufs=3) as prodp,
            tc.tile_pool(name="pbsb", bufs=3) as pbsbp,
            tc.tile_pool(name="smalls", bufs=3) as smalls,
            tc.tile_pool(name="outp", bufs=3) as outp,
            tc.tile_pool(name="projps", bufs=3, space="PSUM") as projps,
            tc.tile_pool(name="scps", bufs=2, space="PSUM") as scps,
            tc.tile_pool(name="pbps", bufs=2, space="PSUM") as pbps,
            tc.tile_pool(name="ptps", bufs=1, space="PSUM") as ptps,
        ):
            # ---- constant + weight + input loads ----
            selrw = consts.tile([128, T * 128], BF16)
            nc.gpsimd.dma_start(out=selrw, in_=selrw_ext[:])
            rwrep = consts.tile([128, T * 128], BF16)
            nc.gpsimd.dma_start(out=rwrep, in_=rwrep_ext[:])
            mergeT = consts.tile([128, ncmb * 128], BF16)
            nc.gpsimd.dma_start(out=mergeT, in_=merge_ext[:])
            ident = consts.tile([128, 128], BF16)
            nc.gpsimd.dma_start(out=ident, in_=ident_ext[:])
            if use_bq:
                bqp = consts.tile([128, KCH], F32)
                nc.gpsimd.dma_start(out=bqp, in_=bq_ext[:])
            if use_bo:
                boe = consts.tile([128, KCH], F32)
                nc.gpsimd.dma_start(out=boe, in_=bo_ext[:])

            wq_t = wpool.tile([128, KCH, D], BF16, tag="w")
            nc.gpsimd.dma_start(
                out=wq_t, in_=wq_ext.rearrange("(k p) o -> p k o", p=128)
            )
            wk_t = wpool.tile([128, KCH, D], BF16, tag="w")
            nc.gpsimd.dma_start(
                out=wk_t, in_=wk_ext.rearrange("(k p) o -> p k o", p=128)
            )

            # embT[p, k, tb] — host pre-transposed; split loads per (k, half)
            embT = embp.tile([128, KCH, TB], BF16)
            for half in range(2):
                for k in range(KCH):
                    eng = nc.sync if (k % 2 == 0) else nc.scalar
                    eng.dma_start(
                        out=embT[:, k, half * 1024 : (half + 1) * 1024],
                        in_=emb_ext[k, :, half * 1024 : (half + 1) * 1024],
                    )

            qT = qp.tile([128, KCH, TB], BF16)
            kT = kp.tile([128, KCH, TB], BF16, tag="kc")
            vT = vp.tile([128, KCH, TB], BF16)
            e_all = attnp.tile([128, T, 128], BF16)
            P_all = attnp.tile([128, T, 128], BF16)
            pt_all = attnp.tile([128, T, 128], BF16)

            def proj_cstep(w_t, dst_all, half, c, bias_t):
                ps0 = projps.tile([128, 512], F32, tag="pp", name="pp")
                ps1 = projps.tile([128, 512], F32, tag="pp", name="pp")
                base = half * 1024
                for k in range(KCH):
                    lhsT = w_t[:, k, c * 128 : (c + 1) * 128]
                    fl = dict(start=(k == 0), stop=(k == KCH - 1))
                    nc.tensor.matmul(ps0, lhsT, embT[:, k, base : base + 512], **fl)
                    nc.tensor.matmul(
                        ps1, lhsT, embT[:, k, base + 512 : base + 1024], **fl
                    )
                d0 = dst_all[:, c, base : base + 512]
                d1 = dst_all[:, c, base + 512 : base + 1024]
                if bias_t is not None:
                    nc.scalar.activation(d0, ps0, AF.Identity, bias=bias_t[:, c])
                    nc.scalar.activation(d1, ps1, AF.Identity, bias=bias_t[:, c])
                else:
                    nc.scalar.copy(out=d0, in_=ps0)
                    nc.scalar.copy(out=d1, in_=ps1)

            def emit_pair(t, j, s0, m):
                """score row-block for pair (t,j): DVE fold + 1 PE matmul."""
                prod = prodp.tile([128, KCH * 128], BF16, tag="prod")
                pv = prod.rearrange("p (k b) -> p k b", b=128)
                nc.vector.tensor_mul(
                    pv,
                    qT[:, :, t * 128 : (t + 1) * 128],
                    kT[:, :, j * 128 : (j + 1) * 128],
                )
                nc.vector.tensor_add(
                    prod[:, 0:512], prod[:, 0:512], prod[:, 512:1024]
                )
                nc.vector.tensor_add(prod[:, 0:256], prod[:, 0:256], prod[:, 256:512])
                nc.vector.tensor_add(prod[:, 0:128], prod[:, 0:128], prod[:, 128:256])
                sc = scps.tile([128, 48], F32, tag="sc", name="sc")
                nc.tensor.matmul(
                    sc[:, 0 : m * 16],
                    prod[:, 0:128],
                    selrw[:, t * 128 + s0 * 16 : t * 128 + (s0 + m) * 16],
                    start=True,
                    stop=True,
                )
                nc.scalar.activation(
                    e_all[:, t, s0 * 16 : (s0 + m) * 16], sc[:, 0 : m * 16], AF.Exp
                )

            def emit_chain(t):
                """softmax normalize + P transpose for table t."""
                ssum = smalls.tile([128, H], F32, tag="ssum")
                nc.vector.reduce_sum(
                    out=ssum,
                    in_=e_all[:, t, :].rearrange("p (s h) -> p h s", h=H),
                    axis=AX.X,
                )
                inv = smalls.tile([128, H], F32, tag="inv")
                nc.vector.reciprocal_approx_fast(out=inv, in_=ssum)
                # P = e * rw(slot) * inv(head)
                nc.vector.tensor_mul(
                    P_all[:, t, :], e_all[:, t, :], rwrep[:, t * 128 : (t + 1) * 128]
                )
                nc.vector.tensor_mul(
                    P_all[:, t, :].rearrange("p (s h) -> p s h", h=H),
                    P_all[:, t, :].rearrange("p (s h) -> p s h", h=H),
                    _bcast_free(inv[:], R, 0),
                )
                ptp = ptps.tile([128, 128], BF16, tag="pt", name="pt")
                nc.tensor.transpose(ptp, P_all[:, t, :], ident)
                nc.scalar.copy(out=pt_all[:, t, :], in_=ptp)

            ctx_first = set()

            def emit_ctx(t, j, s0, m, ctx_all):
                """ctx[t] += merged/broadcast P ⊙ vT[j]."""
                idx = merge_idx[(s0, m)]
                pb = pbps.tile([128, 128], F32, tag="pb", name="pb")
                nc.tensor.matmul(
                    pb,
                    mergeT[:, idx * 128 : (idx + 1) * 128],
                    pt_all[:, t, :],
                    start=True,
                    stop=True,
                )
                pbs = pbsbp.tile([128, 128], BF16, tag="pbs")
                nc.scalar.copy(out=pbs, in_=pb)
                vs = vT[:, :, j * 128 : (j + 1) * 128]
                dst = ctx_all[:, :, t * 128 : (t + 1) * 128]
                if t not in ctx_first:
                    ctx_first.add(t)
                    nc.vector.tensor_mul(dst, vs, _bcast_free(pbs[:], KCH, 0))
                else:
                    tmp = prodp.tile([128, KCH * 128], BF16, tag="prod")
                    tv = tmp.rearrange("p (k b) -> p k b", b=128)
                    nc.vector.tensor_mul(tv, vs, _bcast_free(pbs[:], KCH, 0))
                    nc.vector.tensor_add(dst, dst, tv)

            def emit_o_costep(wo_t, ctx_all, g, co):
                ps = projps.tile([128, 512], F32, tag="pp", name="pp")
                for ci in range(KCH):
                    nc.tensor.matmul(
                        ps,
                        wo_t[:, ci, co * 128 : (co + 1) * 128],
                        ctx_all[:, ci, g * 512 : (g + 1) * 512],
                        start=(ci == 0),
                        stop=(ci == KCH - 1),
                    )
                ob = outp.tile([128, 512], F32)
                if use_bo:
                    nc.scalar.activation(ob, ps, AF.Identity, bias=boe[:, co])
                else:
                    nc.scalar.copy(out=ob, in_=ps)
                nc.sync.dma_start(
                    out=out_ext[co, :, g * 512 : (g + 1) * 512], in_=ob
                )

            # ---------- wave scheduler ----------
            q_done = [False, False]
            k_done = [False, False]
            v_done = [False, False]
            pairs_left_of_t = {t: 0 for t in range(T)}
            for (t, j, s0, m) in pairs:
                pairs_left_of_t[t] += 1
            chain_done = set()
            all_pairs_emitted = [False]
            todo_pairs = list(pairs)
            todo_ctx = []  # filled as chains complete
            ctx_left_of_t = {t: 0 for t in range(T)}
            for (t, j, s0, m) in pairs:
                ctx_left_of_t[t] += 1
            ctx_all_holder = [None]

            def pair_ready(p):
                t, j, s0, m = p
                return q_done[t // 8] and k_done[j // 8]

            def ctx_ready(c):
                t, j, s0, m = c
                return (
                    ctx_all_holder[0] is not None
                    and t in chain_done
                    and v_done[j // 8]
                )

            def flush(budget):
                n = 0
                i = 0
                while i < len(todo_pairs) and n < budget:
                    p = todo_pairs[i]
                    if pair_ready(p):
                        todo_pairs.pop(i)
                        emit_pair(*p)
                        n += 1
                        t = p[0]
                        pairs_left_of_t[t] -= 1
                        if pairs_left_of_t[t] == 0:
                            emit_chain(t)
                            chain_done.add(t)
                            for c in pairs:
                                if c[0] == t:
                                    todo_ctx.append(c)
                    else:
                        i += 1
                if not todo_pairs:
                    all_pairs_emitted[0] = True
                    if ctx_all_holder[0] is None:
                        # qT/kT have no readers after the last pair: the ctx
                        # accumulator can take over kT's SBUF slot now
                        ctx_tile = kp.tile([128, KCH, TB], BF16, tag="kc", name="ctx")
                        ctx_all_holder[0] = ctx_tile
                        todo_ctx.sort(key=lambda c: c[0])
                i = 0
                while i < len(todo_ctx) and n < budget:
                    c = todo_ctx[i]
                    if ctx_ready(c):
                        todo_ctx.pop(i)
                        emit_ctx(*c, ctx_all_holder[0])
                        n += 1
                        ctx_left_of_t[c[0]] -= 1
                    else:
                        i += 1
                return n

            # ---------- main schedule ----------
            phase_list = [
                ("q", 0, qT), ("k", 0, kT), ("q", 1, qT), ("k", 1, kT),
                ("v", 0, vT), ("v", 1, vT),
            ]
            wv_t = wo_t = None
            for (pname, half, dst_all) in phase_list:
                w_t = {"q": wq_t, "k": wk_t, "v": wv_t}[pname]
                bias_t = bqp if (pname == "q" and use_bq) else None
                for c in range(KCH):
                    proj_cstep(w_t, dst_all, half, c, bias_t)
                    flush(6 if pname == "v" else 3)
                if pname == "q":
                    q_done[half] = True
                    if half == 1:
                        # wq dead -> prefetch wv into its slot (overlaps k,1)
                        wv_t = wpool.tile([128, KCH, D], BF16, tag="w")
                        nc.gpsimd.dma_start(
                            out=wv_t,
                            in_=wv_ext.rearrange("(k p) o -> p k o", p=128),
                        )
                elif pname == "k":
                    k_done[half] = True
                    if half == 1:
                        # wk dead -> prefetch wo into its slot (overlaps v)
                        wo_t = wpool.tile([128, KCH, D], BF16, tag="w")
                        nc.gpsimd.dma_start(
                            out=wo_t,
                            in_=wo_ext.rearrange("(k p) o -> p k o", p=128),
                        )
                else:
                    v_done[half] = True

            # drain any remaining score pairs (flush allocates ctx_all when
            # the last one goes out)
            while todo_pairs:
                if flush(8) == 0:
                    raise RuntimeError("scheduler stuck draining pairs")
            ctx_all = ctx_all_holder[0]
            for g in range(T // 4):
                tables = range(4 * g, 4 * g + 4)
                guard = 0
                while any(ctx_left_of_t[t] > 0 for t in tables):
                    made = flush(6)
                    guard += 1
                    if made == 0 and guard > 1000:
                        raise RuntimeError(
                            f"scheduler stuck: g={g} "
                            f"{[(t, ctx_left_of_t[t]) for t in tables]}"
                        )
                for co in range(KCH):
                    emit_o_costep(wo_t, ctx_all, g, co)
                    flush(3)
            while todo_pairs or todo_ctx:
                if flush(8) == 0:
                    raise RuntimeError("scheduler stuck at tail")

    return nc


_CACHE = {}


def _get_program(rel_idx, use_bq, use_bo):
    key = (rel_idx.tobytes(), use_bq, use_bo)
    if key not in _CACHE:
        nc = _build(rel_idx, use_bq, use_bo)
        nc.finalize()
        _CACHE[key] = nc
    return _CACHE[key]


def kernel(
    table_embs,
    rel_embs,
    rel_idx,
    Wq,
    bq,
    Wk,
    bk,
    Wv,
    bv,
    Wo,
    bo,
    w_rel,
    b_rel,
    _trace=False,
):
    table_embs = np.asarray(table_embs, dtype=np.float32)
    rel_embs = np.asarray(rel_embs, dtype=np.float32)
    rel_idx = np.asarray(rel_idx).astype(np.int64)
    Wq, Wk, Wv, Wo = (np.asarray(w, dtype=np.float32) for w in (Wq, Wk, Wv, Wo))
    bq, bk, bv, bo = (np.asarray(b, dtype=np.float32) for b in (bq, bk, bv, bo))
    w_rel = np.asarray(w_rel, dtype=np.float32)
    b_rel = np.asarray(b_rel, dtype=np.float32)

    pairs, slot_r, merge_idx = _structure(rel_idx)
    ncmb = len(merge_idx)

    # ---- host-side prep ----
    rw = 1.0 / (1.0 + np.exp(-(rel_embs @ w_rel + b_rel[0])))  # [T, R] fp32
    bf = ml_dtypes.bfloat16
    wq_p = np.ascontiguousarray(Wq.T[:, _PERM], dtype=bf)
    wk_p = np.ascontiguousarray(Wk.T[:, _PERM], dtype=bf)
    wv_p = np.ascontiguousarray(Wv.T[:, _PERM], dtype=bf)
    wo_p = np.ascontiguousarray(Wo.T[_PERM, :], dtype=bf)

    # selrw[p, t*128 + s*16+h] = 0.125*rw[t, slot_r[t,s]] * (p%16==h)
    # rwrep[p, t*128 + s*16+h] = rw[t, slot_r[t,s]]
    rw_slot = np.take_along_axis(rw, slot_r, axis=1)  # [T, S=R]
    pmod = np.arange(128) % 16
    onehot = (pmod[:, None] == np.arange(16)[None, :]).astype(np.float32)  # [128,16]
    selrw = np.zeros((128, T, R, 16), np.float32)
    rwrep = np.zeros((128, T, R, 16), np.float32)
    for t in range(T):
        for s in range(R):
            selrw[:, t, s, :] = 0.125 * rw_slot[t, s] * onehot
            rwrep[:, t, s, :] = rw_slot[t, s]
    selrw = np.ascontiguousarray(selrw.reshape(128, T * 128), dtype=bf)
    rwrep = np.ascontiguousarray(rwrep.reshape(128, T * 128), dtype=bf)

    # mergeT[p=(s,h), idx*128 + i] = 1{s0<=s<s0+m} * 1{i%16 == h}
    mrg = np.zeros((128, ncmb, 128), np.float32)
    smod = np.arange(128) // 16  # s of partition
    hmod = np.arange(128) % 16  # h of partition
    for (s0, m), idx in merge_idx.items():
        mask = ((smod >= s0) & (smod < s0 + m)).astype(np.float32)  # [128]
        mrg[:, idx, :] = mask[:, None] * (hmod[:, None] == pmod[None, :])
    mrg = np.ascontiguousarray(mrg.reshape(128, ncmb * 128), dtype=bf)
    ident = np.eye(128, dtype=bf)

    use_bq = bool(np.any(bq))
    bo_eff = Wo @ bv + bo
    use_bo = bool(np.any(bo_eff))
    bqp = np.ascontiguousarray(bq[_PERM].reshape(KCH, 128).T, dtype=np.float32)
    boe = np.ascontiguousarray(bo_eff.reshape(KCH, 128).T, dtype=np.float32)

    nc = _get_program(rel_idx, use_bq, use_bo)

    in_maps = []
    for c in range(NCORES):
        e = table_embs[:, c * BC : (c + 1) * BC, :]  # [T, BC, D]
        embT = np.ascontiguousarray(
            e.transpose(2, 0, 1).reshape(KCH, 128, TB), dtype=bf
        )
        m = {
            "emb": embT,
            "wq": wq_p,
            "wk": wk_p,
            "wv": wv_p,
            "wo": wo_p,
            "selrw": selrw,
            "rwrep": rwrep,
            "mrg": mrg,
            "ident": ident,
        }
        if use_bq:
            m["bqp"] = bqp
        if use_bo:
            m["boe"] = boe
        in_maps.append(m)

    res = run_bass_kernel_spmd(nc, in_maps, list(range(NCORES)), trace=_trace)
    out = np.empty((T, B, D), dtype=np.float32)
    for c in range(NCORES):
        o = res.results[c]["out"]  # [KCH, 128, TB]
        out[:, c * BC : (c + 1) * BC, :] = (
            o.reshape(D, T, BC).transpose(1, 2, 0)
        )
    if _trace:
        kernel._last_results = res
    return out
